# revision 1
# baseline (speedup 1.0000x reference)
"""Trainium2 Bass kernel for nn_Decoder (512-step LSTM scan, B=256, F=256).

Strategy: data-parallel over batch across 8 NeuronCores (32 batch/core).
After step 0 the LSTM input equals the hidden state, so W_ih+W_hh fold into
one combined weight for steps 1..511. Step 0 (and the initial_layer) runs on
host in numpy; each core runs 511 uniform recurrence steps.

Per-step device dataflow (batch-major [32, *] tiles):
  PE   : gates = hT.T @ wcT (+bias via ones-row matmul), fp32r, 6 MMs
  ACT  : sigma(i,f), sigma(o), tanh(g), tanh(c2)
  DVE  : c2 = sig_f*c + sig_i*tanh_g ; h2 = sig_o*tanh(c2)
  PE   : transpose h2 -> hT for the next step's stationary operand
"""
import sys

sys.path.insert(0, "/opt/trn_rl_repo")

import numpy as np

SEQ_LEN = 512
B, L, F = 256, 128, 256
NCORES = 8
BS = B // NCORES  # 32 batch per core

_CACHE = {}
VERSION = 3  # bump on every program change: forces a distinct NEFF cache key


def _sigmoid(x):
    out = np.empty_like(x)
    pos = x >= 0
    out[pos] = 1.0 / (1.0 + np.exp(-x[pos]))
    e = np.exp(x[~pos])
    out[~pos] = e / (1.0 + e)
    return out


def _build(steps):
    """Build + schedule the per-core Bass program (same program all cores)."""
    import concourse.mybir as mybir
    import concourse.tile as tile
    from concourse import bacc
    from concourse.masks import make_identity

    f32 = mybir.dt.float32
    f32r = mybir.dt.float32r
    AF = mybir.ActivationFunctionType

    nc = bacc.Bacc("TRN2", target_bir_lowering=False, debug=False)

    hT0_d = nc.dram_tensor("hT0", [F, BS], f32, kind="ExternalInput")
    c1_d = nc.dram_tensor("c1", [BS, F], f32, kind="ExternalInput")
    wcT_d = nc.dram_tensor("wcT", [F, 4 * F], f32, kind="ExternalInput")
    bias_d = nc.dram_tensor("bias", [1, 4 * F], f32, kind="ExternalInput")
    ones_d = nc.dram_tensor("ones", [1, BS], f32, kind="ExternalInput")
    # cache-buster: the neuron NEFF cache key ignores backend_config (the BIR),
    # so distinct programs with identical I/O shapes collide. Unique shape per
    # (VERSION, steps) forces a distinct HLO and cache entry.
    stag_d = nc.dram_tensor("stag", [VERSION, steps], f32, kind="ExternalInput")
    outs_d = nc.dram_tensor("outs", [SEQ_LEN, BS, F], f32, kind="ExternalOutput")

    with tile.TileContext(nc) as tc:
        with tc.tile_pool(name="const", bufs=1) as cpool, \
             tc.tile_pool(name="state", bufs=2) as spool, \
             tc.tile_pool(name="work", bufs=2) as wpool, \
             tc.tile_pool(name="h2p", bufs=6) as hpool, \
             tc.tile_pool(name="ps", bufs=2, space="PSUM") as psp:

            wc_sb = cpool.tile([128, 2 * 4 * F], f32r)
            nc.gpsimd.dma_start(
                out=wc_sb[:].rearrange("p (k n) -> p k n", k=2),
                in_=wcT_d.ap().rearrange("(k p) n -> p k n", p=128))
            bias_sb = cpool.tile([1, 4 * F], f32r)
            nc.gpsimd.dma_start(out=bias_sb[:], in_=bias_d.ap())
            ones_sb = cpool.tile([1, BS], f32r)
            nc.gpsimd.dma_start(out=ones_sb[:], in_=ones_d.ap())
            ident = cpool.tile([32, 32], f32)
            make_identity(nc, ident)
            stag_sb = cpool.tile([1, 1], f32)
            nc.sync.dma_start(out=stag_sb[:], in_=stag_d.ap()[0:1, 0:1])

            hT_cur = spool.tile([128, 2 * BS], f32r, tag="hT")
            nc.gpsimd.dma_start(
                out=hT_cur[:].rearrange("p (k b) -> p k b", k=2),
                in_=hT0_d.ap().rearrange("(k p) b -> p k b", p=128))
            c_cur = spool.tile([BS, F], f32, tag="c")
            nc.sync.dma_start(out=c_cur[:], in_=c1_d.ap())

            for t in range(1, steps + 1):
                psA = psp.tile([BS, 512], f32, tag="psA")
                psB = psp.tile([BS, 512], f32, tag="psB")
                for ps, off in ((psA, 0), (psB, 512)):
                    for k in range(2):
                        nc.tensor.matmul(
                            ps[:], lhsT=hT_cur[:, BS * k:BS * (k + 1)],
                            rhs=wc_sb[:, 4 * F * k + off: 4 * F * k + off + 512],
                            start=(k == 0), stop=False)
                    nc.tensor.matmul(
                        ps[:], lhsT=ones_sb[:], rhs=bias_sb[:, off:off + 512],
                        start=False, stop=True)

                sA = wpool.tile([BS, 512], f32, tag="sA")
                nc.scalar.activation(sA[:], psA[:], AF.Sigmoid)
                sO = wpool.tile([BS, F], f32, tag="sO")
                nc.scalar.activation(sO[:], psB[:, 0:F], AF.Sigmoid)
                tg = wpool.tile([BS, F], f32, tag="tg")
                nc.scalar.activation(tg[:], psB[:, F:2 * F], AF.Tanh)

                t2 = wpool.tile([BS, F], f32, tag="t2")
                nc.vector.tensor_mul(t2[:], sA[:, F:2 * F], c_cur[:])
                t1 = wpool.tile([BS, F], f32, tag="t1")
                nc.vector.tensor_mul(t1[:], sA[:, 0:F], tg[:])
                c_new = spool.tile([BS, F], f32, tag="c")
                nc.vector.tensor_add(c_new[:], t1[:], t2[:])
                tc_t = wpool.tile([BS, F], f32, tag="tc")
                nc.scalar.activation(tc_t[:], c_new[:], AF.Tanh)
                h2 = hpool.tile([BS, F], f32, tag="h2")
                nc.vector.tensor_mul(h2[:], sO[:], tc_t[:])

                nc.sync.dma_start(out=outs_d.ap()[t], in_=h2[:])

                if t < steps:
                    tps = psp.tile([128, 2 * BS], f32, tag="tps")
                    for k in range(2):
                        nc.tensor.transpose(
                            tps[:, BS * k:BS * (k + 1)],
                            h2[:, 128 * k:128 * (k + 1)], ident[:])
                    hT_new = spool.tile([128, 2 * BS], f32r, tag="hT")
                    nc.scalar.copy(hT_new[:], tps[:])
                    hT_cur = hT_new
                c_cur = c_new

    nc.compile()
    return nc


def _get_nc(steps):
    if steps not in _CACHE:
        _CACHE[steps] = _build(steps)
    return _CACHE[steps]


def _host_prep(x, last_feat, Wi, bi, W_ih, W_hh, b_ih, b_hh):
    x = np.asarray(x, np.float32)
    last_feat = np.asarray(last_feat, np.float32)
    Wi = np.asarray(Wi, np.float32); bi = np.asarray(bi, np.float32)
    W_ih = np.asarray(W_ih, np.float32); W_hh = np.asarray(W_hh, np.float32)
    b_ih = np.asarray(b_ih, np.float32); b_hh = np.asarray(b_hh, np.float32)

    z = x[0] @ Wi.T + bi                       # [B, F]
    init = np.where(z > 0, z, np.expm1(z)).astype(np.float32)  # elu

    bsum = b_ih + b_hh
    g0 = last_feat @ W_ih.T + init @ W_hh.T + bsum   # [B, 4F] order i,f,g,o
    i0, f0, g0g, o0 = (g0[:, 0:F], g0[:, F:2*F], g0[:, 2*F:3*F], g0[:, 3*F:4*F])
    c1 = _sigmoid(f0) * init + _sigmoid(i0) * np.tanh(g0g)
    h1 = (_sigmoid(o0) * np.tanh(c1)).astype(np.float32)
    c1 = c1.astype(np.float32)

    # combined recurrent weight, rows reordered [i, f, o, g]
    Wc = W_ih + W_hh                            # [4F, F]
    perm = np.concatenate([np.arange(0, F), np.arange(F, 2*F),
                           np.arange(3*F, 4*F), np.arange(2*F, 3*F)])
    wcT = np.ascontiguousarray(Wc[perm].T)      # [F, 4F] gate order i,f,o,g
    bias_row = np.ascontiguousarray(bsum[perm][None, :])  # [1, 4F]
    return h1, c1, wcT, bias_row


_steps_of = [SEQ_LEN - 1]


def _in_maps(inputs, steps=None):
    _steps_of[0] = steps or _steps_of[0]
    h1, c1, wcT, bias_row = _host_prep(
        inputs["x"], inputs["last_feat"], inputs["Wi"], inputs["bi"],
        inputs["W_ih"], inputs["W_hh"], inputs["b_ih"], inputs["b_hh"])
    ones = np.ones((1, BS), np.float32)
    maps = []
    for ci in range(NCORES):
        s = slice(ci * BS, (ci + 1) * BS)
        maps.append(dict(
            hT0=np.ascontiguousarray(h1[s].T),
            c1=np.ascontiguousarray(c1[s]),
            wcT=wcT, bias=bias_row, ones=ones,
            stag=np.zeros((VERSION, _steps_of[0]), np.float32)))
    return maps


def kernel(x, last_feat, Wi, bi, W_ih, W_hh, b_ih, b_hh, Wo, bo,
           _steps=SEQ_LEN - 1):
    from concourse.bass_utils import run_bass_kernel_spmd

    h1, c1, wcT, bias_row = _host_prep(x, last_feat, Wi, bi, W_ih, W_hh,
                                       b_ih, b_hh)
    ones = np.ones((1, BS), np.float32)
    in_maps = []
    for ci in range(NCORES):
        s = slice(ci * BS, (ci + 1) * BS)
        in_maps.append(dict(
            hT0=np.ascontiguousarray(h1[s].T),
            c1=np.ascontiguousarray(c1[s]),
            wcT=wcT, bias=bias_row, ones=ones))

    for m in in_maps:
        m["stag"] = np.zeros((VERSION, _steps), np.float32)
    nc = _get_nc(_steps)
    res = run_bass_kernel_spmd(nc, in_maps, core_ids=list(range(NCORES)))

    outs = np.concatenate([r["outs"] for r in res.results], axis=1)  # [S, B, F]
    outs[0] = h1
    return np.ascontiguousarray(outs).reshape(B, SEQ_LEN, F)



# revision 20
# speedup vs baseline: 575.9204x; 575.9204x over previous
"""Trainium2 Bass kernel for nn_Decoder (512-step LSTM scan, B=256, F=256).

Strategy: data-parallel over batch across 8 NeuronCores (32 batch/core).
After step 0 the LSTM input equals the hidden state, so W_ih+W_hh fold into
one combined weight for steps 1..511. Step 0 (and the initial_layer) runs on
host in numpy; each core runs 511 uniform recurrence steps.

v5 feature-major redesign: state h,c live as [128, 2, 32] (feature on
partitions). Gates are computed feature-major with stationary weight chunks
(16 accumulating matmuls, fp32 exact), bias is folded into the activation
instructions (per-partition bias AP), and the recurrence needs NO transpose:
h_new is directly the next step's moving operand. Gate chunk order g,f,i,o
lets tanh(g)/sigmoid(f) start while i/o matmuls still run. The batch-major
copy of h for the output DMA is produced by 2 PE transposes + ACT copy off
the critical path.
"""
import sys

sys.path.insert(0, "/opt/trn_rl_repo")

import numpy as np

SEQ_LEN = 512
B, L, F = 256, 128, 256
NCORES = 8
BS = B // NCORES  # 32 batch per core

_CACHE = {}
VERSION = 7  # bump on every program change: forces a distinct NEFF cache key


def _sigmoid(x):
    out = np.empty_like(x)
    pos = x >= 0
    out[pos] = 1.0 / (1.0 + np.exp(-x[pos]))
    e = np.exp(x[~pos])
    out[~pos] = e / (1.0 + e)
    return out


def _build(steps, repeat=1):
    """Build + schedule the per-core Bass program (same program all cores).

    repeat>1 wraps the whole scan in a hardware For_i loop re-running the
    identical scan; used only for device-time measurement (wall(R2)-wall(R1)
    isolates device execution from per-call dispatch/transfer overhead).
    """
    import concourse.mybir as mybir
    import concourse.tile as tile
    from concourse import bacc
    from concourse.masks import make_identity

    f32 = mybir.dt.float32
    AF = mybir.ActivationFunctionType

    nc = bacc.Bacc("TRN2", target_bir_lowering=False, debug=False)

    # feature-major initial state: [128, 2, 32] = h[b, 128k+p] at [p, k, b]
    h0_d = nc.dram_tensor("h0", [128, 2 * BS], f32, kind="ExternalInput")
    c0_d = nc.dram_tensor("c0", [128, 2 * BS], f32, kind="ExternalInput")
    # stationary weight chunks: wst[:, (2m+k)*128:(2m+k+1)*128] = lhsT for
    # gate-chunk m, feature-chunk k (gate order g,f,i,o)
    wst_d = nc.dram_tensor("wst", [128, 16 * 128], f32, kind="ExternalInput")
    # bias rows for the K=2 selector matmul: bsel[j, 128g:128g+128] = bias of
    # gate g, feature-chunk j; sel[j, 32k:32k+32] = (j == k)
    bsel_d = nc.dram_tensor("bsel", [2, 4 * 128], f32, kind="ExternalInput")
    sel_d = nc.dram_tensor("sel", [2, 2 * BS], f32, kind="ExternalInput")
    # cache-buster: the neuron NEFF cache key ignores backend_config, so
    # distinct programs with identical I/O shapes collide. Unique shape per
    # (VERSION, steps, repeat) forces a distinct HLO and cache entry.
    stag_d = nc.dram_tensor("stag", [VERSION, steps, repeat], f32,
                            kind="ExternalInput")
    outs_d = nc.dram_tensor("outs", [SEQ_LEN, BS, F], f32, kind="ExternalOutput")

    with tile.TileContext(nc) as tc:
        with tc.tile_pool(name="const", bufs=1) as cpool, \
             tc.tile_pool(name="state", bufs=3) as spool, \
             tc.tile_pool(name="work", bufs=2) as wpool, \
             tc.tile_pool(name="h2p", bufs=4) as hpool, \
             tc.tile_pool(name="ps", bufs=2, space="PSUM") as psp, \
             tc.tile_pool(name="psg", bufs=1, space="PSUM") as psgp:

            wst_sb = cpool.tile([128, 16 * 128], f32)
            nc.gpsimd.dma_start(out=wst_sb[:], in_=wst_d.ap())
            bsel_sb = cpool.tile([2, 4 * 128], f32)
            nc.gpsimd.dma_start(out=bsel_sb[:], in_=bsel_d.ap())
            sel_sb = cpool.tile([2, 2 * BS], f32)
            nc.gpsimd.dma_start(out=sel_sb[:], in_=sel_d.ap())
            ident = cpool.tile([128, 128], f32)
            make_identity(nc, ident)
            stag_sb = cpool.tile([1, 1], f32)
            nc.sync.dma_start(out=stag_sb[:], in_=stag_d.ap()[0:1, 0:1, 0])

            from concourse.tile import add_dep_helper

            def emit_out(h_tile, t, pe_anchor, dve_anchor):
                # batch-major copy of h(t) for the output DMA. Order-only deps
                # keep the recurrence first in each engine queue: transposes
                # run after the next step's matmul burst, and the PSUM->SBUF
                # copy runs after the next h update (the greedy dispatcher
                # would otherwise slot the 392ns copy into the 100ns sem
                # window before c_new becomes runnable).
                tps = psp.tile([BS, 2 * 128], f32, tag="tps")
                for k in range(2):
                    tp_i = nc.tensor.transpose(
                        tps[:, 128 * k:128 * (k + 1)],
                        h_tile[:, BS * k:BS * (k + 1)], ident[:])
                    if pe_anchor is not None:
                        add_dep_helper(pe_anchor.ins, tp_i.ins, sync=False,
                                       reason="recurrence MMs first on PE")
                h2bm = hpool.tile([BS, F], f32, tag="h2bm")
                cp_i = nc.vector.tensor_copy(h2bm[:], tps[:])
                if dve_anchor is not None:
                    add_dep_helper(dve_anchor.ins, cp_i.ins, sync=False,
                                   reason="recurrence muls first on DVE")
                nc.sync.dma_start(out=outs_d.ap()[t], in_=h2bm[:])

            def scan_body():
                h_first = spool.tile([128, 2 * BS], f32, tag="h")
                nc.sync.dma_start(out=h_first[:], in_=h0_d.ap())
                c_first = spool.tile([128, 2 * BS], f32, tag="c")
                nc.sync.dma_start(out=c_first[:], in_=c0_d.ap())
                h_cur, c_cur = h_first, c_first
                pending = None  # (h_tile, t) waiting for its output store

                for t in range(1, steps + 1):
                    # one PSUM bank per gate (f, g, i, o); bias lands first
                    # via a K=2 selector matmul (start=True clears the bank),
                    # then 4 accumulating weight matmuls per gate.
                    psg = []
                    for g in range(4):
                        psg_t = psgp.tile([128, 2 * BS], f32, tag=f"ps{g}",
                                          name=f"psg{g}")
                        psg.append(psg_t)
                    last_mm = None
                    for g in range(4):
                        nc.tensor.matmul(
                            psg[g][:], lhsT=bsel_sb[:, g * 128:(g + 1) * 128],
                            rhs=sel_sb[:], start=True, stop=False)
                        for j in range(2):
                            for k in range(2):
                                m = 2 * g + j
                                last_mm = nc.tensor.matmul(
                                    psg[g][:, BS * j:BS * (j + 1)],
                                    lhsT=wst_sb[:, (2 * m + k) * 128:(2 * m + k + 1) * 128],
                                    rhs=h_cur[:, BS * k:BS * (k + 1)],
                                    start=False, stop=(j == 1 and k == 1))

                    # one activation op per gate; ACT order f -> g -> i -> o
                    SF = wpool.tile([128, 2 * BS], f32, tag="SF")
                    TG = wpool.tile([128, 2 * BS], f32, tag="TG")
                    SI = wpool.tile([128, 2 * BS], f32, tag="SI")
                    SO = wpool.tile([128, 2 * BS], f32, tag="SO")
                    for dst, fn, g in ((SF, AF.Sigmoid, 0), (TG, AF.Tanh, 1),
                                       (SI, AF.Sigmoid, 2), (SO, AF.Sigmoid, 3)):
                        nc.scalar.activation(dst[:], psg[g][:], fn)

                    # c_new = SF*c + SI*TG ; h_new = SO * tanh(c_new)
                    t2 = wpool.tile([128, 2 * BS], f32, tag="t2")
                    nc.vector.tensor_mul(t2[:], SF[:], c_cur[:])
                    t1 = wpool.tile([128, 2 * BS], f32, tag="t1")
                    nc.vector.tensor_mul(t1[:], SI[:], TG[:])
                    c_new = spool.tile([128, 2 * BS], f32, tag="c")
                    nc.vector.tensor_add(c_new[:], t1[:], t2[:])
                    tch = wpool.tile([128, 2 * BS], f32, tag="tch")
                    nc.scalar.activation(tch[:], c_new[:], AF.Tanh)
                    h_new = spool.tile([128, 2 * BS], f32, tag="h")
                    hmul = nc.vector.tensor_mul(h_new[:], SO[:], tch[:])

                    if pending is not None:
                        emit_out(*pending, pe_anchor=last_mm, dve_anchor=hmul)
                    pending = (h_new, t)
                    h_cur, c_cur = h_new, c_new

                emit_out(*pending, pe_anchor=None, dve_anchor=None)

            if repeat == 1:
                scan_body()
            else:
                with tc.For_i(0, repeat, 1):
                    scan_body()

    nc.compile()
    return nc


def _get_nc(steps, repeat=1):
    key = (steps, repeat)
    if key not in _CACHE:
        _CACHE[key] = _build(steps, repeat)
    return _CACHE[key]


def _host_prep(x, last_feat, Wi, bi, W_ih, W_hh, b_ih, b_hh):
    x = np.asarray(x, np.float32)
    last_feat = np.asarray(last_feat, np.float32)
    Wi = np.asarray(Wi, np.float32); bi = np.asarray(bi, np.float32)
    W_ih = np.asarray(W_ih, np.float32); W_hh = np.asarray(W_hh, np.float32)
    b_ih = np.asarray(b_ih, np.float32); b_hh = np.asarray(b_hh, np.float32)

    z = x[0] @ Wi.T + bi                       # [B, F]
    init = np.where(z > 0, z, np.expm1(z)).astype(np.float32)  # elu

    bsum = b_ih + b_hh
    g0 = last_feat @ W_ih.T + init @ W_hh.T + bsum   # [B, 4F] order i,f,g,o
    i0, f0, g0g, o0 = (g0[:, 0:F], g0[:, F:2*F], g0[:, 2*F:3*F], g0[:, 3*F:4*F])
    c1 = _sigmoid(f0) * init + _sigmoid(i0) * np.tanh(g0g)
    h1 = (_sigmoid(o0) * np.tanh(c1)).astype(np.float32)
    c1 = c1.astype(np.float32)

    # combined recurrent weight, rows reordered [f, g, i, o]
    Wc = (W_ih + W_hh).astype(np.float32)       # [4F, F] gate order i,f,g,o
    perm = np.concatenate([np.arange(F, 2*F), np.arange(2*F, 3*F),
                           np.arange(0, F), np.arange(3*F, 4*F)])
    Wp = Wc[perm]                                # [4F=8*128, F] order f,g,i,o
    bp = bsum[perm].astype(np.float32)
    # stationary chunks: wst[:, (2m+k)*128:...] = Wp[128m:128m+128, 128k:128k+128].T
    wst = np.empty((128, 16 * 128), np.float32)
    for m in range(8):
        for k in range(2):
            wst[:, (2*m+k)*128:(2*m+k+1)*128] = \
                Wp[128*m:128*(m+1), 128*k:128*(k+1)].T
    bsel = np.ascontiguousarray(bp.reshape(4, 2, 128).transpose(1, 0, 2)
                                .reshape(2, 4 * 128))  # [2, 4*128]
    return h1, c1, wst, bsel


def _to_fm(a):
    """[BS, F] batch-major -> [128, 2*BS] feature-major."""
    return np.ascontiguousarray(
        a.T.reshape(2, 128, BS).transpose(1, 0, 2).reshape(128, 2 * BS))


def _in_maps(inputs, steps, repeat=1):
    h1, c1, wst, bsel = _host_prep(
        inputs["x"], inputs["last_feat"], inputs["Wi"], inputs["bi"],
        inputs["W_ih"], inputs["W_hh"], inputs["b_ih"], inputs["b_hh"])
    stag = np.zeros((VERSION, steps, repeat), np.float32)
    sel = np.zeros((2, 2 * BS), np.float32)
    sel[0, :BS] = 1.0
    sel[1, BS:] = 1.0
    maps = []
    for ci in range(NCORES):
        s = slice(ci * BS, (ci + 1) * BS)
        maps.append(dict(
            h0=_to_fm(h1[s]), c0=_to_fm(c1[s]),
            wst=wst, bsel=bsel, sel=sel, stag=stag))
    return maps, h1


def kernel(x, last_feat, Wi, bi, W_ih, W_hh, b_ih, b_hh, Wo, bo,
           _steps=SEQ_LEN - 1, _repeat=1):
    from concourse.bass_utils import run_bass_kernel_spmd

    inputs = dict(x=x, last_feat=last_feat, Wi=Wi, bi=bi, W_ih=W_ih,
                  W_hh=W_hh, b_ih=b_ih, b_hh=b_hh)
    in_maps, h1 = _in_maps(inputs, _steps, _repeat)
    nc = _get_nc(_steps, _repeat)
    res = run_bass_kernel_spmd(nc, in_maps, core_ids=list(range(NCORES)))

    outs = np.concatenate([r["outs"] for r in res.results], axis=1)  # [S, B, F]
    outs[0] = h1
    return np.ascontiguousarray(outs).reshape(B, SEQ_LEN, F)


# revision 21
# speedup vs baseline: 614.0220x; 1.0662x over previous
"""Trainium2 Bass kernel for nn_Decoder (512-step LSTM scan, B=256, F=256).

Strategy: data-parallel over batch across 8 NeuronCores (32 batch/core).
After step 0 the LSTM input equals the hidden state, so W_ih+W_hh fold into
one combined weight for steps 1..511. Step 0 (and the initial_layer) runs on
host in numpy; each core runs 511 uniform recurrence steps.

v5 feature-major redesign: state h,c live as [128, 2, 32] (feature on
partitions). Gates are computed feature-major with stationary weight chunks
(16 accumulating matmuls, fp32 exact), bias is folded into the activation
instructions (per-partition bias AP), and the recurrence needs NO transpose:
h_new is directly the next step's moving operand. Gate chunk order g,f,i,o
lets tanh(g)/sigmoid(f) start while i/o matmuls still run. The batch-major
copy of h for the output DMA is produced by 2 PE transposes + ACT copy off
the critical path.
"""
import sys

sys.path.insert(0, "/opt/trn_rl_repo")

import numpy as np

SEQ_LEN = 512
B, L, F = 256, 128, 256
NCORES = 8
BS = B // NCORES  # 32 batch per core

_CACHE = {}
VERSION = 7  # bump on every program change: forces a distinct NEFF cache key


def _sigmoid(x):
    out = np.empty_like(x)
    pos = x >= 0
    out[pos] = 1.0 / (1.0 + np.exp(-x[pos]))
    e = np.exp(x[~pos])
    out[~pos] = e / (1.0 + e)
    return out


def _build(steps, repeat=1):
    """Build + schedule the per-core Bass program (same program all cores).

    repeat>1 wraps the whole scan in a hardware For_i loop re-running the
    identical scan; used only for device-time measurement (wall(R2)-wall(R1)
    isolates device execution from per-call dispatch/transfer overhead).
    """
    import concourse.mybir as mybir
    import concourse.tile as tile
    from concourse import bacc
    from concourse.masks import make_identity

    f32 = mybir.dt.float32
    AF = mybir.ActivationFunctionType

    nc = bacc.Bacc("TRN2", target_bir_lowering=False, debug=False)

    # feature-major initial state: [128, 2, 32] = h[b, 128k+p] at [p, k, b]
    h0_d = nc.dram_tensor("h0", [128, 2 * BS], f32, kind="ExternalInput")
    c0_d = nc.dram_tensor("c0", [128, 2 * BS], f32, kind="ExternalInput")
    # stationary weight chunks: wst[:, (2m+k)*128:(2m+k+1)*128] = lhsT for
    # gate-chunk m, feature-chunk k (gate order g,f,i,o)
    wst_d = nc.dram_tensor("wst", [128, 16 * 128], f32, kind="ExternalInput")
    # bias rows for the K=2 selector matmul: bsel[j, 128g:128g+128] = bias of
    # gate g, feature-chunk j; sel[j, 32k:32k+32] = (j == k)
    bsel_d = nc.dram_tensor("bsel", [2, 4 * 128], f32, kind="ExternalInput")
    sel_d = nc.dram_tensor("sel", [2, 2 * BS], f32, kind="ExternalInput")
    # cache-buster: the neuron NEFF cache key ignores backend_config, so
    # distinct programs with identical I/O shapes collide. Unique shape per
    # (VERSION, steps, repeat) forces a distinct HLO and cache entry.
    stag_d = nc.dram_tensor("stag", [VERSION, steps, repeat], f32,
                            kind="ExternalInput")
    outs_d = nc.dram_tensor("outs", [SEQ_LEN, BS, F], f32, kind="ExternalOutput")

    with tile.TileContext(nc) as tc:
        with tc.tile_pool(name="const", bufs=1) as cpool, \
             tc.tile_pool(name="state", bufs=3) as spool, \
             tc.tile_pool(name="work", bufs=2) as wpool, \
             tc.tile_pool(name="h2p", bufs=4) as hpool, \
             tc.tile_pool(name="ps", bufs=2, space="PSUM") as psp, \
             tc.tile_pool(name="psg", bufs=1, space="PSUM") as psgp:

            wst_sb = cpool.tile([128, 16 * 128], f32)
            nc.gpsimd.dma_start(out=wst_sb[:], in_=wst_d.ap())
            bsel_sb = cpool.tile([2, 4 * 128], f32)
            nc.gpsimd.dma_start(out=bsel_sb[:], in_=bsel_d.ap())
            sel_sb = cpool.tile([2, 2 * BS], f32)
            nc.gpsimd.dma_start(out=sel_sb[:], in_=sel_d.ap())
            ident = cpool.tile([128, 128], f32)
            make_identity(nc, ident)
            stag_sb = cpool.tile([1, 1], f32)
            nc.sync.dma_start(out=stag_sb[:], in_=stag_d.ap()[0:1, 0:1, 0])

            from concourse.tile import add_dep_helper

            def emit_out(h_tile, t, pe_anchor, dve_anchor):
                # batch-major copy of h(t) for the output DMA. Order-only deps
                # keep the recurrence first in each engine queue: transposes
                # run after the next step's matmul burst, and the PSUM->SBUF
                # copy runs after the next h update (the greedy dispatcher
                # would otherwise slot the 392ns copy into the 100ns sem
                # window before c_new becomes runnable).
                tps = psp.tile([BS, 2 * 128], f32, tag="tps")
                for k in range(2):
                    tp_i = nc.tensor.transpose(
                        tps[:, 128 * k:128 * (k + 1)],
                        h_tile[:, BS * k:BS * (k + 1)], ident[:])
                    if pe_anchor is not None:
                        add_dep_helper(pe_anchor.ins, tp_i.ins, sync=False,
                                       reason="recurrence MMs first on PE")
                h2bm = hpool.tile([BS, F], f32, tag="h2bm")
                # chunked so a greedily-slotted piece steals <=130ns from the
                # critical DVE chain (a single 392ns copy kept landing in the
                # sem window before t1/c_new)
                for q in range(4):
                    cp_i = nc.vector.tensor_copy(
                        h2bm[:, 64 * q:64 * (q + 1)], tps[:, 64 * q:64 * (q + 1)])
                    if dve_anchor is not None:
                        add_dep_helper(dve_anchor.ins, cp_i.ins, sync=False,
                                       reason="recurrence muls first on DVE")
                nc.sync.dma_start(out=outs_d.ap()[t], in_=h2bm[:])

            def scan_body():
                h_first = spool.tile([128, 2 * BS], f32, tag="h")
                nc.sync.dma_start(out=h_first[:], in_=h0_d.ap())
                c_first = spool.tile([128, 2 * BS], f32, tag="c")
                nc.sync.dma_start(out=c_first[:], in_=c0_d.ap())
                h_cur, c_cur = h_first, c_first
                pending = None  # (h_tile, t) waiting for its output store

                for t in range(1, steps + 1):
                    # one PSUM bank per gate (f, g, i, o); bias lands first
                    # via a K=2 selector matmul (start=True clears the bank),
                    # then 4 accumulating weight matmuls per gate.
                    psg = []
                    for g in range(4):
                        psg_t = psgp.tile([128, 2 * BS], f32, tag=f"ps{g}",
                                          name=f"psg{g}")
                        psg.append(psg_t)
                    last_mm = None
                    for g in range(4):
                        nc.tensor.matmul(
                            psg[g][:], lhsT=bsel_sb[:, g * 128:(g + 1) * 128],
                            rhs=sel_sb[:], start=True, stop=False)
                        for j in range(2):
                            for k in range(2):
                                m = 2 * g + j
                                last_mm = nc.tensor.matmul(
                                    psg[g][:, BS * j:BS * (j + 1)],
                                    lhsT=wst_sb[:, (2 * m + k) * 128:(2 * m + k + 1) * 128],
                                    rhs=h_cur[:, BS * k:BS * (k + 1)],
                                    start=False, stop=(j == 1 and k == 1))

                    # one activation op per gate; ACT order f -> g -> i -> o
                    SF = wpool.tile([128, 2 * BS], f32, tag="SF")
                    TG = wpool.tile([128, 2 * BS], f32, tag="TG")
                    SI = wpool.tile([128, 2 * BS], f32, tag="SI")
                    SO = wpool.tile([128, 2 * BS], f32, tag="SO")
                    for dst, fn, g in ((SF, AF.Sigmoid, 0), (TG, AF.Tanh, 1),
                                       (SI, AF.Sigmoid, 2), (SO, AF.Sigmoid, 3)):
                        nc.scalar.activation(dst[:], psg[g][:], fn)

                    # c_new = SF*c + SI*TG ; h_new = SO * tanh(c_new)
                    t2 = wpool.tile([128, 2 * BS], f32, tag="t2")
                    nc.vector.tensor_mul(t2[:], SF[:], c_cur[:])
                    t1 = wpool.tile([128, 2 * BS], f32, tag="t1")
                    nc.vector.tensor_mul(t1[:], SI[:], TG[:])
                    c_new = spool.tile([128, 2 * BS], f32, tag="c")
                    nc.vector.tensor_add(c_new[:], t1[:], t2[:])
                    tch = wpool.tile([128, 2 * BS], f32, tag="tch")
                    nc.scalar.activation(tch[:], c_new[:], AF.Tanh)
                    h_new = spool.tile([128, 2 * BS], f32, tag="h")
                    hmul = nc.vector.tensor_mul(h_new[:], SO[:], tch[:])

                    if pending is not None:
                        emit_out(*pending, pe_anchor=last_mm, dve_anchor=hmul)
                    pending = (h_new, t)
                    h_cur, c_cur = h_new, c_new

                emit_out(*pending, pe_anchor=None, dve_anchor=None)

            if repeat == 1:
                scan_body()
            else:
                with tc.For_i(0, repeat, 1):
                    scan_body()

    nc.compile()
    return nc


def _get_nc(steps, repeat=1):
    key = (steps, repeat)
    if key not in _CACHE:
        _CACHE[key] = _build(steps, repeat)
    return _CACHE[key]


def _host_prep(x, last_feat, Wi, bi, W_ih, W_hh, b_ih, b_hh):
    x = np.asarray(x, np.float32)
    last_feat = np.asarray(last_feat, np.float32)
    Wi = np.asarray(Wi, np.float32); bi = np.asarray(bi, np.float32)
    W_ih = np.asarray(W_ih, np.float32); W_hh = np.asarray(W_hh, np.float32)
    b_ih = np.asarray(b_ih, np.float32); b_hh = np.asarray(b_hh, np.float32)

    z = x[0] @ Wi.T + bi                       # [B, F]
    init = np.where(z > 0, z, np.expm1(z)).astype(np.float32)  # elu

    bsum = b_ih + b_hh
    g0 = last_feat @ W_ih.T + init @ W_hh.T + bsum   # [B, 4F] order i,f,g,o
    i0, f0, g0g, o0 = (g0[:, 0:F], g0[:, F:2*F], g0[:, 2*F:3*F], g0[:, 3*F:4*F])
    c1 = _sigmoid(f0) * init + _sigmoid(i0) * np.tanh(g0g)
    h1 = (_sigmoid(o0) * np.tanh(c1)).astype(np.float32)
    c1 = c1.astype(np.float32)

    # combined recurrent weight, rows reordered [f, g, i, o]
    Wc = (W_ih + W_hh).astype(np.float32)       # [4F, F] gate order i,f,g,o
    perm = np.concatenate([np.arange(F, 2*F), np.arange(2*F, 3*F),
                           np.arange(0, F), np.arange(3*F, 4*F)])
    Wp = Wc[perm]                                # [4F=8*128, F] order f,g,i,o
    bp = bsum[perm].astype(np.float32)
    # stationary chunks: wst[:, (2m+k)*128:...] = Wp[128m:128m+128, 128k:128k+128].T
    wst = np.empty((128, 16 * 128), np.float32)
    for m in range(8):
        for k in range(2):
            wst[:, (2*m+k)*128:(2*m+k+1)*128] = \
                Wp[128*m:128*(m+1), 128*k:128*(k+1)].T
    bsel = np.ascontiguousarray(bp.reshape(4, 2, 128).transpose(1, 0, 2)
                                .reshape(2, 4 * 128))  # [2, 4*128]
    return h1, c1, wst, bsel


def _to_fm(a):
    """[BS, F] batch-major -> [128, 2*BS] feature-major."""
    return np.ascontiguousarray(
        a.T.reshape(2, 128, BS).transpose(1, 0, 2).reshape(128, 2 * BS))


def _in_maps(inputs, steps, repeat=1):
    h1, c1, wst, bsel = _host_prep(
        inputs["x"], inputs["last_feat"], inputs["Wi"], inputs["bi"],
        inputs["W_ih"], inputs["W_hh"], inputs["b_ih"], inputs["b_hh"])
    stag = np.zeros((VERSION, steps, repeat), np.float32)
    sel = np.zeros((2, 2 * BS), np.float32)
    sel[0, :BS] = 1.0
    sel[1, BS:] = 1.0
    maps = []
    for ci in range(NCORES):
        s = slice(ci * BS, (ci + 1) * BS)
        maps.append(dict(
            h0=_to_fm(h1[s]), c0=_to_fm(c1[s]),
            wst=wst, bsel=bsel, sel=sel, stag=stag))
    return maps, h1


def kernel(x, last_feat, Wi, bi, W_ih, W_hh, b_ih, b_hh, Wo, bo,
           _steps=SEQ_LEN - 1, _repeat=1):
    from concourse.bass_utils import run_bass_kernel_spmd

    inputs = dict(x=x, last_feat=last_feat, Wi=Wi, bi=bi, W_ih=W_ih,
                  W_hh=W_hh, b_ih=b_ih, b_hh=b_hh)
    in_maps, h1 = _in_maps(inputs, _steps, _repeat)
    nc = _get_nc(_steps, _repeat)
    res = run_bass_kernel_spmd(nc, in_maps, core_ids=list(range(NCORES)))

    outs = np.concatenate([r["outs"] for r in res.results], axis=1)  # [S, B, F]
    outs[0] = h1
    return np.ascontiguousarray(outs).reshape(B, SEQ_LEN, F)
